# revision 5
# baseline (speedup 1.0000x reference)
"""BCE + weighted Dice loss on 8 Trainium2 NeuronCores.

Full inputs logits/targets [4,3,128,128,128] f32 are sharded along the depth
axis D=128 into 8 slices of 16. Each core reduces its shard to a small set of
per-(b,c) partial sums; the host combines them into the scalar loss.

Math notes (s := sigmoid(-x)):
  sigmoid(x)   = 1 - s
  softplus(x)  = -ln(s)
  sum(prob)    = N - sum(s)
  sum(prob*t)  = sum(t) - sum(s*t)
  bce_sum      = sum(softplus(x)) - sum(x*t) = -sum(ln s) - sum(x*t)
  pred         = (x >= 0.5);  t*pred == (u >= 0.5) with u = x*t (t in {0,1})

Per-core device outputs:
  stats_act [128, 36]: per-partition accums from ScalarE: 3 cols per slab
      (sum s, sum sigmoid(-u) [ACT-variant slabs only], sum ln s)
  stats_dve [128, 48]: per-partition accums from VectorE: 4 cols per slab
      (sum x*t, sum pred, sum t*pred, sum s*t [DVE-variant slabs only])
  tsum [12, 512]: PE ones-matmul column sums of t per slab (sum over the 512
      columns gives sum(t) for that (b,c) slab)
"""

import sys

if "/opt/trn_rl_repo" not in sys.path:
    sys.path.insert(0, "/opt/trn_rl_repo")

import numpy as np

import concourse.bacc as bacc
import concourse.mybir as mybir
from concourse import tile
from concourse.alu_op_type import AluOpType
from concourse.bass_utils import run_bass_kernel_spmd

# Problem geometry (hardcoded per harness contract).
B, C, D, H, W = 4, 3, 128, 128, 128
N_CORES = 8
D_SHARD = D // N_CORES            # 16
SLABS = B * C                     # 12 (b,c) slabs per core
P = 128                           # SBUF partitions
F = D_SHARD * H * W // P          # 2048 free elems per partition
N_SLAB = P * F                    # 262144 elems per core-slab
N_TOTAL = B * C * D * H * W

# First A_SLABS slabs route sum(s*t) through ScalarE (sigmoid(-u) + correction),
# the rest through VectorE (scalar_tensor_tensor). Balances ACT vs DVE load.
A_SLABS = 9

_CACHED = {}


def _build():
    if "nc" in _CACHED:
        return _CACHED["nc"]
    AFT = mybir.ActivationFunctionType
    f32 = mybir.dt.float32
    bf16 = mybir.dt.bfloat16

    nc = bacc.Bacc("TRN2", target_bir_lowering=False, debug=False,
                   num_devices=N_CORES)
    x_d = nc.dram_tensor("logits", [SLABS, P, F], f32, kind="ExternalInput")
    t_d = nc.dram_tensor("targets", [SLABS, P, F], f32, kind="ExternalInput")
    sa_d = nc.dram_tensor("stats_act", [P, 3 * SLABS], f32, kind="ExternalOutput")
    sd_d = nc.dram_tensor("stats_dve", [P, 4 * SLABS], f32, kind="ExternalOutput")
    ts_d = nc.dram_tensor("tsum", [P, 4 * 512], f32, kind="ExternalOutput")

    with tile.TileContext(nc) as tc:
        with (
            tc.tile_pool(name="xt", bufs=3) as xt_pool,
            tc.tile_pool(name="s", bufs=SLABS) as s_pool,
            tc.tile_pool(name="scr", bufs=2) as scr_pool,
            tc.tile_pool(name="misc", bufs=1) as misc_pool,
            tc.tile_pool(name="psum", bufs=1, space="PSUM") as psum_pool,
        ):
            stats_act = misc_pool.tile([P, 3 * SLABS], f32)
            stats_dve = misc_pool.tile([P, 4 * SLABS], f32)
            nc.vector.memset(stats_act[:], 0.0)
            nc.vector.memset(stats_dve[:], 0.0)
            ones = misc_pool.tile([P, 1], f32)
            nc.vector.memset(ones[:], 1.0)
            # 12 slab rows packed 3-per-PSUM-bank at partitions {0,32,64}
            pt_banks = [psum_pool.tile([P, 512], f32, name=f"pt{b}",
                                       tag=f"pt{b}") for b in range(4)]

            s_tiles = []
            # ---- Phase A: everything except ln(s); sigmoid table set ----
            for s_i in range(SLABS):
                x = xt_pool.tile([P, F], f32, tag="x")
                t = xt_pool.tile([P, F], f32, tag="t")
                nc.sync.dma_start(x[:], x_d[s_i])
                nc.sync.dma_start(t[:], t_d[s_i])

                # s = sigmoid(-x); accum -> sum(s)
                s = s_pool.tile([P, F], bf16, tag="s")
                s_tiles.append(s)
                nc.scalar.activation(
                    s[:], x[:], AFT.Sigmoid, scale=-1.0,
                    accum_out=stats_act[:, 3 * s_i:3 * s_i + 1],
                )

                # u = x*t; accum -> sum(x*t)
                u = scr_pool.tile([P, F], f32, tag="u")
                nc.vector.scalar_tensor_tensor(
                    out=u[:], in0=x[:], scalar=1.0, in1=t[:],
                    op0=AluOpType.mult, op1=AluOpType.mult,
                    accum_out=stats_dve[:, 4 * s_i:4 * s_i + 1],
                )
                # pred = (x >= 0.5); accum -> sum(pred)
                pr = scr_pool.tile([P, F], bf16, tag="scr16")
                nc.vector.tensor_scalar(
                    out=pr[:], in0=x[:], scalar1=0.5, scalar2=0.0,
                    op0=AluOpType.is_ge, op1=AluOpType.add,
                    accum_out=stats_dve[:, 4 * s_i + 1:4 * s_i + 2],
                )
                # t*pred = (u >= 0.5); accum -> sum(t*pred)
                tp = scr_pool.tile([P, F], bf16, tag="scr16")
                nc.vector.tensor_scalar(
                    out=tp[:], in0=u[:], scalar1=0.5, scalar2=0.0,
                    op0=AluOpType.is_ge, op1=AluOpType.add,
                    accum_out=stats_dve[:, 4 * s_i + 2:4 * s_i + 3],
                )

                if s_i < A_SLABS:
                    # sum(sigmoid(-u)) on ScalarE; host corrects to sum(s*t)
                    w = scr_pool.tile([P, F], bf16, tag="scr16")
                    nc.scalar.activation(
                        w[:], u[:], AFT.Sigmoid, scale=-1.0,
                        accum_out=stats_act[:, 3 * s_i + 1:3 * s_i + 2],
                    )
                else:
                    # sum(s*t) directly on VectorE
                    st = scr_pool.tile([P, F], f32, tag="u")
                    nc.vector.scalar_tensor_tensor(
                        out=st[:], in0=s[:], scalar=1.0, in1=t[:],
                        op0=AluOpType.mult, op1=AluOpType.mult,
                        accum_out=stats_dve[:, 4 * s_i + 3:4 * s_i + 4],
                    )

                # sum(t) columns via PE: ones[128,1].T @ t chunks -> psum row
                bank, row = s_i // 3, (s_i % 3) * 32
                for j in range(4):
                    nc.tensor.matmul(
                        pt_banks[bank][row:row + 1, :],
                        ones[:], t[:, j * 512:(j + 1) * 512],
                        start=(j == 0), stop=(j == 3),
                    )

            # ---- Phase B: ln(s) accums; natural_log table set ----
            for s_i in range(SLABS):
                l = scr_pool.tile([P, F], bf16, tag="scr16")
                nc.scalar.activation(
                    l[:], s_tiles[s_i][:], AFT.Ln,
                    accum_out=stats_act[:, 3 * s_i + 2:3 * s_i + 3],
                )

            # ---- Epilogue: move partials out ----
            ts_sb = misc_pool.tile([P, 4 * 512], f32)
            for b in range(4):
                nc.scalar.copy(ts_sb[:, b * 512:(b + 1) * 512], pt_banks[b][:])
            nc.sync.dma_start(ts_d[:], ts_sb[:])
            nc.sync.dma_start(sa_d[:], stats_act[:])
            nc.sync.dma_start(sd_d[:], stats_dve[:])

    nc.compile()
    _CACHED["nc"] = nc
    return nc


def _shard_inputs(logits: np.ndarray, targets: np.ndarray):
    """Shard along D into 8 contiguous [SLABS, P, F] blocks."""
    in_maps = []
    for i in range(N_CORES):
        sl = slice(i * D_SHARD, (i + 1) * D_SHARD)
        x = np.ascontiguousarray(logits[:, :, sl]).reshape(SLABS, P, F)
        t = np.ascontiguousarray(targets[:, :, sl]).reshape(SLABS, P, F)
        in_maps.append({"logits": x, "targets": t})
    return in_maps


def _combine(results):
    """Host-side reduction of per-core partials to the scalar loss."""
    EPS = 1e-9
    S_s = np.zeros(SLABS)
    S_l = np.zeros(SLABS)
    S_xt = np.zeros(SLABS)
    S_pred = np.zeros(SLABS)
    S_tp = np.zeros(SLABS)
    S_st = np.zeros(SLABS)
    S_t = np.zeros(SLABS)
    for r in results:
        sa = r["stats_act"].astype(np.float64)
        sd = r["stats_dve"].astype(np.float64)
        ts = r["tsum"].astype(np.float64)
        for s_i in range(SLABS):
            t_sum = ts[(s_i % 3) * 32, (s_i // 3) * 512:(s_i // 3 + 1) * 512].sum()
            S_t[s_i] += t_sum
            S_s[s_i] += sa[:, 3 * s_i].sum()
            S_l[s_i] += sa[:, 3 * s_i + 2].sum()
            S_xt[s_i] += sd[:, 4 * s_i].sum()
            S_pred[s_i] += sd[:, 4 * s_i + 1].sum()
            S_tp[s_i] += sd[:, 4 * s_i + 2].sum()
            if s_i < A_SLABS:
                # sum(s*t) = sum(sigmoid(-u)) - 0.5*(N_slab - sum(t))
                S_st[s_i] += sa[:, 3 * s_i + 1].sum() - 0.5 * (N_SLAB - t_sum)
            else:
                S_st[s_i] += sd[:, 4 * s_i + 3].sum()

    sum_prob = N_TOTAL - S_s.sum()
    sum_pt = S_t.sum() - S_st.sum()          # sum(prob * t)
    sum_sp = -S_l.sum()                      # sum(softplus(x))
    bce = (sum_sp - S_xt.sum()) / N_TOTAL

    union = sum_prob + S_t.sum()
    inter = 2.0 * sum_pt
    dice_loss = 1.0 - (inter + EPS) / union

    score = np.where(
        (S_t == 0) & (S_pred == 0),
        np.ones_like(S_t),
        (2.0 * S_tp + EPS) / (S_t + S_pred),
    ).reshape(B, C)
    per_class = score.mean(axis=0)

    loss = (bce + dice_loss * 0.5 + per_class[0] * 0.2
            + per_class[1] * 0.1 + per_class[2] * 0.2)
    return np.float32(loss)


def kernel(logits: np.ndarray, targets: np.ndarray) -> np.ndarray:
    nc = _build()
    in_maps = _shard_inputs(np.asarray(logits), np.asarray(targets))
    res = run_bass_kernel_spmd(nc, in_maps, list(range(N_CORES)))
    return _combine(res.results)


# revision 6
# speedup vs baseline: 1.4894x; 1.4894x over previous
"""BCE + weighted Dice loss on 8 Trainium2 NeuronCores.

Full inputs logits/targets [4,3,128,128,128] f32 are sharded along the depth
axis D=128 into 8 slices of 16 and converted to bf16 on the host (halves DMA;
targets are {0,1} so exact, logits rounding shifts the loss by ~1e-5 rel).
Each core reduces its shard to per-(b,c) partial sums; the host combines them.

Math notes (s := sigmoid(-x)):
  sigmoid(x)   = 1 - s
  softplus(x)  = -ln(s)
  sum(prob)    = N - sum(s)
  sum(prob*t)  = sum(t) - sum(s*t)
  bce_sum      = -sum(ln s) - sum(x*t)
  pred         = (x >= 0.5);  t*pred = t*(x>=0.5)

Engine split per (b,c) slab tile [128, 2048] bf16:
  ScalarE: s = sigmoid(-x) (+accum sum s), later ln(s) (+accum sum ln s)
  VectorE: pred = (x >= 0.5) -> bf16 (4x mode), PSUM diag-mask extractions
  TensorE: diagonal-trick matmuls for sum(x*t), sum(s*t), sum(t*pred);
           ones-matmuls for per-slab sum(t), sum(pred)

The diagonal trick: accumulating chunk matmuls A[:,c128].T @ B[:,c128] into
one PSUM bank leaves sum_c sum_p A[p,cm]*B[p,cn] at [m,n]; the diagonal
m == n carries the elementwise dot product. Masking by the identity and
summing recovers sum(A*B) without any slow DVE reduce.

Device outputs per core:
  stats_act [128, 24]: ScalarE accums (2 cols/slab: sum s, sum ln s)
  diag_xt, diag_st [128, 128] f32: masked global-diag accumulators; their
      total sum is sum(x*t) resp. sum(s*t) over the whole shard
  diag_tp [12, 128, 128] f32: per-slab masked accumulators; total sum of
      entry s is that slab's sum(t*pred)
  trows [96, 2048] f32: ones-matmul row banks; slab s row lives at
      partition (s%3)*32, cols 1024*q + 512*(s//6) + 256*((s//3)%2) for
      quantity q in {0: sum(t), 1: sum(pred)}, 256 wide
"""

import sys

if "/opt/trn_rl_repo" not in sys.path:
    sys.path.insert(0, "/opt/trn_rl_repo")

import numpy as np

import concourse.bacc as bacc
import concourse.mybir as mybir
from concourse import tile
from concourse.alu_op_type import AluOpType
from concourse.bass_utils import run_bass_kernel_spmd

# Problem geometry (hardcoded per harness contract).
B, C, D, H, W = 4, 3, 128, 128, 128
N_CORES = 8
D_SHARD = D // N_CORES            # 16
SLABS = B * C                     # 12 (b,c) slabs per core
P = 128                           # SBUF partitions
F = D_SHARD * H * W // P          # 2048 free elems per partition
N_SLAB = P * F                    # 262144 elems per core-slab
N_TOTAL = B * C * D * H * W
NCH = F // 128                    # 16 diag chunks per slab

_CACHED = {}


def _build():
    if "nc" in _CACHED:
        return _CACHED["nc"]
    AFT = mybir.ActivationFunctionType
    f32 = mybir.dt.float32
    bf16 = mybir.dt.bfloat16

    nc = bacc.Bacc("TRN2", target_bir_lowering=False, debug=False,
                   num_devices=N_CORES)
    x_d = nc.dram_tensor("logits", [SLABS, P, F], bf16, kind="ExternalInput")
    t_d = nc.dram_tensor("targets", [SLABS, P, F], bf16, kind="ExternalInput")
    id_d = nc.dram_tensor("ident", [P, 128], bf16, kind="ExternalInput")
    sa_d = nc.dram_tensor("stats_act", [P, 2 * SLABS], f32, kind="ExternalOutput")
    dxt_d = nc.dram_tensor("diag_xt", [P, 128], f32, kind="ExternalOutput")
    dst_d = nc.dram_tensor("diag_st", [P, 128], f32, kind="ExternalOutput")
    dtp_d = nc.dram_tensor("diag_tp", [SLABS, P, 128], f32, kind="ExternalOutput")
    tr_d = nc.dram_tensor("trows", [96, 2048], f32, kind="ExternalOutput")

    with tile.TileContext(nc) as tc:
        with (
            tc.tile_pool(name="xt", bufs=4) as xt_pool,
            tc.tile_pool(name="s", bufs=SLABS) as s_pool,
            tc.tile_pool(name="scr", bufs=2) as scr_pool,
            tc.tile_pool(name="misc", bufs=1) as misc_pool,
            tc.tile_pool(name="psum", bufs=1, space="PSUM") as psum_pool,
        ):
            stats_act = misc_pool.tile([P, 2 * SLABS], f32)
            nc.vector.memset(stats_act[:], 0.0)
            ones = misc_pool.tile([P, 1], bf16)
            nc.vector.memset(ones[:], 1.0)
            ident = misc_pool.tile([P, 128], bf16)
            nc.sync.dma_start(ident[:], id_d[:])

            # PSUM banks (8 total): 2 global diag accumulators, 2 rotating
            # per-slab tp accumulators, 2+2 row banks for sum(t)/sum(pred).
            p_xt = psum_pool.tile([P, 128], f32, name="p_xt", tag="p_xt")
            p_st = psum_pool.tile([P, 128], f32, name="p_st", tag="p_st")
            p_tp = [psum_pool.tile([P, 128], f32, name=f"p_tp{i}", tag=f"p_tp{i}")
                    for i in range(2)]
            p_t = [psum_pool.tile([P, 512], f32, name=f"p_t{i}", tag=f"p_t{i}")
                   for i in range(2)]
            p_pr = [psum_pool.tile([P, 512], f32, name=f"p_pr{i}", tag=f"p_pr{i}")
                    for i in range(2)]

            s_tiles = []
            # ---- Phase A (sigmoid table set resident) ----
            for s_i in range(SLABS):
                xb = xt_pool.tile([P, F], bf16, tag="x")
                tb = xt_pool.tile([P, F], bf16, tag="t")
                nc.sync.dma_start(xb[:], x_d[s_i])
                nc.sync.dma_start(tb[:], t_d[s_i])

                # s = sigmoid(-x) (+ accum sum s)
                s = s_pool.tile([P, F], bf16, tag="s")
                s_tiles.append(s)
                nc.scalar.activation(
                    s[:], xb[:], AFT.Sigmoid, scale=-1.0,
                    accum_out=stats_act[:, 2 * s_i:2 * s_i + 1],
                )
                # pred = (x >= 0.5) in bf16 (4x mode)
                pred = scr_pool.tile([P, F], bf16, tag="pred")
                nc.vector.tensor_scalar(
                    out=pred[:], in0=xb[:], scalar1=0.5, scalar2=None,
                    op0=AluOpType.is_ge,
                )

                first = s_i == 0
                last = s_i == SLABS - 1
                for c in range(NCH):
                    sl = slice(c * 128, (c + 1) * 128)
                    nc.tensor.matmul(p_xt[:, :], xb[:, sl], tb[:, sl],
                                     start=(first and c == 0),
                                     stop=(last and c == NCH - 1))
                for c in range(NCH):
                    sl = slice(c * 128, (c + 1) * 128)
                    nc.tensor.matmul(p_st[:, :], s[:, sl], tb[:, sl],
                                     start=(first and c == 0),
                                     stop=(last and c == NCH - 1))
                tp_bank = p_tp[s_i % 2]
                for c in range(NCH):
                    sl = slice(c * 128, (c + 1) * 128)
                    nc.tensor.matmul(tp_bank[:, :], pred[:, sl], tb[:, sl],
                                     start=(c == 0), stop=(c == NCH - 1))
                # Per-slab extraction of the tp diagonal (masked, f32).
                mtp = scr_pool.tile([P, 128], f32, tag="mtp")
                nc.vector.tensor_tensor(out=mtp[:], in0=tp_bank[:, :],
                                        in1=ident[:], op=AluOpType.mult)
                nc.sync.dma_start(dtp_d[s_i], mtp[:])

                # Row sums: sum(t) and sum(pred) per slab via ones-matmuls,
                # 8 chunks of 256 accumulated into one [1,256] row slot.
                row = (s_i % 3) * 32
                colblk = ((s_i // 3) % 2) * 256
                t_bank = p_t[s_i // 6]
                pr_bank = p_pr[s_i // 6]
                for c in range(8):
                    sl = slice(c * 256, (c + 1) * 256)
                    nc.tensor.matmul(t_bank[row:row + 1, colblk:colblk + 256],
                                     ones[:], tb[:, sl],
                                     start=(c == 0), stop=(c == 7))
                for c in range(8):
                    sl = slice(c * 256, (c + 1) * 256)
                    nc.tensor.matmul(pr_bank[row:row + 1, colblk:colblk + 256],
                                     ones[:], pred[:, sl],
                                     start=(c == 0), stop=(c == 7))

            # ---- Phase B: ln(s) accums (natural_log table set) ----
            for s_i in range(SLABS):
                l = scr_pool.tile([P, F], bf16, tag="l")
                nc.scalar.activation(
                    l[:], s_tiles[s_i][:], AFT.Ln,
                    accum_out=stats_act[:, 2 * s_i + 1:2 * s_i + 2],
                )

            # ---- Epilogue ----
            mxt = misc_pool.tile([P, 128], f32)
            nc.vector.tensor_tensor(out=mxt[:], in0=p_xt[:, :], in1=ident[:],
                                    op=AluOpType.mult)
            nc.sync.dma_start(dxt_d[:], mxt[:])
            mst = misc_pool.tile([P, 128], f32)
            nc.vector.tensor_tensor(out=mst[:], in0=p_st[:, :], in1=ident[:],
                                    op=AluOpType.mult)
            nc.sync.dma_start(dst_d[:], mst[:])

            trows = misc_pool.tile([96, 2048], f32)
            for i in range(2):
                nc.scalar.copy(trows[0:96, 512 * i:512 * (i + 1)],
                               p_t[i][0:96, :])
                nc.scalar.copy(trows[0:96, 1024 + 512 * i:1024 + 512 * (i + 1)],
                               p_pr[i][0:96, :])
            nc.sync.dma_start(tr_d[:], trows[:])
            nc.sync.dma_start(sa_d[:], stats_act[:])

    nc.compile()
    _CACHED["nc"] = nc
    return nc


def _to_bf16_bits(a: np.ndarray) -> np.ndarray:
    """f32 -> bf16 bits with round-to-nearest-even, returned as uint16."""
    u = np.ascontiguousarray(a, dtype=np.float32).view(np.uint32)
    rounded = ((u + 0x7FFF + ((u >> 16) & 1)) >> 16).astype(np.uint16)
    return rounded


def _shard_inputs(logits: np.ndarray, targets: np.ndarray):
    import ml_dtypes

    bf = ml_dtypes.bfloat16
    xb = _to_bf16_bits(logits).view(bf)
    tb = _to_bf16_bits(targets).view(bf)
    eye = np.eye(P, 128, dtype=np.float32).astype(bf)
    in_maps = []
    for i in range(N_CORES):
        sl = slice(i * D_SHARD, (i + 1) * D_SHARD)
        x = np.ascontiguousarray(xb[:, :, sl]).reshape(SLABS, P, F)
        t = np.ascontiguousarray(tb[:, :, sl]).reshape(SLABS, P, F)
        in_maps.append({"logits": x, "targets": t, "ident": eye})
    return in_maps


def _combine(results):
    """Host-side reduction of per-core partials to the scalar loss."""
    EPS = 1e-9
    S_s = np.zeros(SLABS)
    S_l = np.zeros(SLABS)
    S_tp = np.zeros(SLABS)
    S_t = np.zeros(SLABS)
    S_pred = np.zeros(SLABS)
    S_xt = 0.0
    S_st = 0.0
    for r in results:
        sa = r["stats_act"].astype(np.float64)
        S_xt += r["diag_xt"].astype(np.float64).sum()
        S_st += r["diag_st"].astype(np.float64).sum()
        tr = r["trows"].astype(np.float64)
        dtp = r["diag_tp"].astype(np.float64)
        for s_i in range(SLABS):
            S_s[s_i] += sa[:, 2 * s_i].sum()
            S_l[s_i] += sa[:, 2 * s_i + 1].sum()
            S_tp[s_i] += dtp[s_i].sum()
            row = (s_i % 3) * 32
            col = 512 * (s_i // 6) + 256 * ((s_i // 3) % 2)
            S_t[s_i] += tr[row, col:col + 256].sum()
            S_pred[s_i] += tr[row, 1024 + col:1024 + col + 256].sum()

    sum_prob = N_TOTAL - S_s.sum()
    sum_pt = S_t.sum() - S_st               # sum(prob * t)
    sum_sp = -S_l.sum()                     # sum(softplus(x))
    bce = (sum_sp - S_xt) / N_TOTAL

    union = sum_prob + S_t.sum()
    inter = 2.0 * sum_pt
    dice_loss = 1.0 - (inter + EPS) / union

    score = np.where(
        (S_t == 0) & (S_pred == 0),
        np.ones_like(S_t),
        (2.0 * S_tp + EPS) / (S_t + S_pred),
    ).reshape(B, C)
    per_class = score.mean(axis=0)

    loss = (bce + dice_loss * 0.5 + per_class[0] * 0.2
            + per_class[1] * 0.1 + per_class[2] * 0.2)
    return np.float32(loss)


def kernel(logits: np.ndarray, targets: np.ndarray) -> np.ndarray:
    nc = _build()
    in_maps = _shard_inputs(np.asarray(logits), np.asarray(targets))
    res = run_bass_kernel_spmd(nc, in_maps, list(range(N_CORES)))
    return _combine(res.results)


# revision 8
# speedup vs baseline: 1.4897x; 1.0002x over previous
"""BCE + weighted Dice loss on 8 Trainium2 NeuronCores.

Full inputs logits/targets [4,3,128,128,128] f32 are sharded along the depth
axis D=128 into 8 slices of 16 and converted to bf16 on the host (halves DMA;
targets are {0,1} so exact, logits rounding shifts the loss by ~1e-5 rel).
Each core reduces its shard to per-(b,c) partial sums; the host combines them.

Math notes (s := sigmoid(-x)):
  sigmoid(x)   = 1 - s
  softplus(x)  = -ln(s)
  sum(prob)    = N - sum(s)
  sum(prob*t)  = sum(t) - sum(s*t)
  bce_sum      = -sum(ln s) - sum(x*t)
  pred         = (x >= 0.5);  t*pred = t*(x>=0.5)

Engine split per (b,c) slab tile [128, 2048] bf16:
  ScalarE: s = sigmoid(-x) (+accum sum s), later ln(s) (+accum sum ln s)
  VectorE: pred = (x >= 0.5) -> bf16 (4x mode), PSUM diag-mask extractions
  TensorE: diagonal-trick matmuls for sum(x*t), sum(s*t), sum(t*pred);
           ones-matmuls for per-slab sum(t), sum(pred)

The diagonal trick: accumulating chunk matmuls A[:,c128].T @ B[:,c128] into
one PSUM bank leaves sum_c sum_p A[p,cm]*B[p,cn] at [m,n]; the diagonal
m == n carries the elementwise dot product. Masking by the identity and
summing recovers sum(A*B) without any slow DVE reduce.

Device outputs per core:
  stats_act [128, 24]: ScalarE accums (2 cols/slab: sum s, sum ln s)
  diag_xt, diag_st [128, 128] f32: masked global-diag accumulators; their
      total sum is sum(x*t) resp. sum(s*t) over the whole shard
  diag_tp [12, 128, 128] f32: per-slab masked accumulators; total sum of
      entry s is that slab's sum(t*pred)
  trows [96, 2048] f32: ones-matmul row banks; slab s row lives at
      partition (s%3)*32, cols 1024*q + 512*(s//6) + 256*((s//3)%2) for
      quantity q in {0: sum(t), 1: sum(pred)}, 256 wide
"""

import sys

if "/opt/trn_rl_repo" not in sys.path:
    sys.path.insert(0, "/opt/trn_rl_repo")

import numpy as np

import concourse.bacc as bacc
import concourse.mybir as mybir
from concourse import tile
from concourse.alu_op_type import AluOpType
from concourse.bass_utils import run_bass_kernel_spmd

# Problem geometry (hardcoded per harness contract).
B, C, D, H, W = 4, 3, 128, 128, 128
N_CORES = 8
D_SHARD = D // N_CORES            # 16
SLABS = B * C                     # 12 (b,c) slabs per core
P = 128                           # SBUF partitions
F = D_SHARD * H * W // P          # 2048 free elems per partition
N_SLAB = P * F                    # 262144 elems per core-slab
N_TOTAL = B * C * D * H * W
NCH = F // 128                    # 16 diag chunks per slab

_CACHED = {}


def _build():
    if "nc" in _CACHED:
        return _CACHED["nc"]
    AFT = mybir.ActivationFunctionType
    f32 = mybir.dt.float32
    bf16 = mybir.dt.bfloat16

    nc = bacc.Bacc("TRN2", target_bir_lowering=False, debug=False,
                   num_devices=N_CORES)
    x_d = nc.dram_tensor("logits", [SLABS, P, F], bf16, kind="ExternalInput")
    t_d = nc.dram_tensor("targets", [SLABS, P, F], bf16, kind="ExternalInput")
    id_d = nc.dram_tensor("ident", [P, 128], bf16, kind="ExternalInput")
    sa_d = nc.dram_tensor("stats_act", [P, 2 * SLABS], f32, kind="ExternalOutput")
    dxt_d = nc.dram_tensor("diag_xt", [P, 128], f32, kind="ExternalOutput")
    dst_d = nc.dram_tensor("diag_st", [P, 128], f32, kind="ExternalOutput")
    dtp_d = nc.dram_tensor("diag_tp", [SLABS, P, 128], f32, kind="ExternalOutput")
    tr_d = nc.dram_tensor("trows", [96, 2048], f32, kind="ExternalOutput")

    with tile.TileContext(nc) as tc:
        with (
            tc.tile_pool(name="xt", bufs=4) as xt_pool,
            tc.tile_pool(name="s", bufs=SLABS) as s_pool,
            tc.tile_pool(name="scr", bufs=2) as scr_pool,
            tc.tile_pool(name="misc", bufs=1) as misc_pool,
            tc.tile_pool(name="psum", bufs=1, space="PSUM") as psum_pool,
        ):
            stats_act = misc_pool.tile([P, 2 * SLABS], f32)
            nc.vector.memset(stats_act[:], 0.0)
            ones = misc_pool.tile([P, 1], bf16)
            nc.vector.memset(ones[:], 1.0)
            ident = misc_pool.tile([P, 128], bf16)
            nc.sync.dma_start(ident[:], id_d[:])

            # PSUM banks (8 total): 2 global diag accumulators, 2 rotating
            # per-slab tp accumulators, 2+2 row banks for sum(t)/sum(pred).
            p_xt = psum_pool.tile([P, 128], f32, name="p_xt", tag="p_xt")
            p_st = psum_pool.tile([P, 128], f32, name="p_st", tag="p_st")
            p_tp = [psum_pool.tile([P, 128], f32, name=f"p_tp{i}", tag=f"p_tp{i}")
                    for i in range(2)]
            p_t = [psum_pool.tile([P, 512], f32, name=f"p_t{i}", tag=f"p_t{i}")
                   for i in range(2)]
            p_pr = [psum_pool.tile([P, 512], f32, name=f"p_pr{i}", tag=f"p_pr{i}")
                    for i in range(2)]

            s_tiles = []
            # ---- Phase A (sigmoid table set resident) ----
            for s_i in range(SLABS):
                xb = xt_pool.tile([P, F], bf16, tag="x")
                tb = xt_pool.tile([P, F], bf16, tag="t")
                nc.sync.dma_start(xb[:], x_d[s_i])
                nc.sync.dma_start(tb[:], t_d[s_i])

                # s = sigmoid(-x) (+ accum sum s)
                s = s_pool.tile([P, F], bf16, tag="s")
                s_tiles.append(s)
                nc.scalar.activation(
                    s[:], xb[:], AFT.Sigmoid, scale=-1.0,
                    accum_out=stats_act[:, 2 * s_i:2 * s_i + 1],
                )
                # pred = (x >= 0.5) in bf16 (4x mode)
                pred = scr_pool.tile([P, F], bf16, tag="pred")
                nc.vector.tensor_scalar(
                    out=pred[:], in0=xb[:], scalar1=0.5, scalar2=None,
                    op0=AluOpType.is_ge,
                )

                first = s_i == 0
                last = s_i == SLABS - 1
                for c in range(NCH):
                    sl = slice(c * 128, (c + 1) * 128)
                    nc.tensor.matmul(p_xt[:, :], xb[:, sl], tb[:, sl],
                                     start=(first and c == 0),
                                     stop=(last and c == NCH - 1))
                for c in range(NCH):
                    sl = slice(c * 128, (c + 1) * 128)
                    nc.tensor.matmul(p_st[:, :], s[:, sl], tb[:, sl],
                                     start=(first and c == 0),
                                     stop=(last and c == NCH - 1))
                tp_bank = p_tp[s_i % 2]
                for c in range(NCH):
                    sl = slice(c * 128, (c + 1) * 128)
                    nc.tensor.matmul(tp_bank[:, :], pred[:, sl], tb[:, sl],
                                     start=(c == 0), stop=(c == NCH - 1))
                # Per-slab extraction of the tp diagonal (masked, f32).
                mtp = scr_pool.tile([P, 128], f32, tag="mtp")
                nc.vector.tensor_tensor(out=mtp[:], in0=tp_bank[:, :],
                                        in1=ident[:], op=AluOpType.mult)
                nc.gpsimd.dma_start(dtp_d[s_i], mtp[:])

                # Row sums: sum(t) and sum(pred) per slab via ones-matmuls,
                # 8 chunks of 256 accumulated into one [1,256] row slot.
                row = (s_i % 3) * 32
                colblk = ((s_i // 3) % 2) * 256
                t_bank = p_t[s_i // 6]
                pr_bank = p_pr[s_i // 6]
                for c in range(8):
                    sl = slice(c * 256, (c + 1) * 256)
                    nc.tensor.matmul(t_bank[row:row + 1, colblk:colblk + 256],
                                     ones[:], tb[:, sl],
                                     start=(c == 0), stop=(c == 7))
                for c in range(8):
                    sl = slice(c * 256, (c + 1) * 256)
                    nc.tensor.matmul(pr_bank[row:row + 1, colblk:colblk + 256],
                                     ones[:], pred[:, sl],
                                     start=(c == 0), stop=(c == 7))

            # ---- Phase B: ln(s) accums (natural_log table set) ----
            # zb is written by a ScalarE op reading the last s tile, and every
            # ln pass reads zb as its bias: this forces the scheduler to place
            # all Ln activations after all Sigmoid ones, so the activation
            # table set switches exactly once instead of thrashing.
            zb = misc_pool.tile([P, 1], f32)
            nc.scalar.activation(zb[:], s_tiles[-1][:, 0:1], AFT.Copy, scale=0.0)
            for s_i in range(SLABS):
                l = scr_pool.tile([P, F], bf16, tag="l")
                nc.scalar.activation(
                    l[:], s_tiles[s_i][:], AFT.Ln, bias=zb[:],
                    accum_out=stats_act[:, 2 * s_i + 1:2 * s_i + 2],
                )

            # ---- Epilogue ----
            mxt = misc_pool.tile([P, 128], f32)
            nc.vector.tensor_tensor(out=mxt[:], in0=p_xt[:, :], in1=ident[:],
                                    op=AluOpType.mult)
            nc.sync.dma_start(dxt_d[:], mxt[:])
            mst = misc_pool.tile([P, 128], f32)
            nc.vector.tensor_tensor(out=mst[:], in0=p_st[:, :], in1=ident[:],
                                    op=AluOpType.mult)
            nc.sync.dma_start(dst_d[:], mst[:])

            trows = misc_pool.tile([96, 2048], f32)
            for i in range(2):
                nc.vector.tensor_copy(trows[0:96, 512 * i:512 * (i + 1)],
                                      p_t[i][0:96, :])
                nc.vector.tensor_copy(trows[0:96, 1024 + 512 * i:1024 + 512 * (i + 1)],
                                      p_pr[i][0:96, :])
            nc.sync.dma_start(tr_d[:], trows[:])
            nc.sync.dma_start(sa_d[:], stats_act[:])

    nc.compile()
    _CACHED["nc"] = nc
    return nc


def _to_bf16_bits(a: np.ndarray) -> np.ndarray:
    """f32 -> bf16 bits with round-to-nearest-even, returned as uint16."""
    u = np.ascontiguousarray(a, dtype=np.float32).view(np.uint32)
    rounded = ((u + 0x7FFF + ((u >> 16) & 1)) >> 16).astype(np.uint16)
    return rounded


def _shard_inputs(logits: np.ndarray, targets: np.ndarray):
    import ml_dtypes

    bf = ml_dtypes.bfloat16
    xb = _to_bf16_bits(logits).view(bf)
    tb = _to_bf16_bits(targets).view(bf)
    eye = np.eye(P, 128, dtype=np.float32).astype(bf)
    in_maps = []
    for i in range(N_CORES):
        sl = slice(i * D_SHARD, (i + 1) * D_SHARD)
        x = np.ascontiguousarray(xb[:, :, sl]).reshape(SLABS, P, F)
        t = np.ascontiguousarray(tb[:, :, sl]).reshape(SLABS, P, F)
        in_maps.append({"logits": x, "targets": t, "ident": eye})
    return in_maps


def _combine(results):
    """Host-side reduction of per-core partials to the scalar loss."""
    EPS = 1e-9
    S_s = np.zeros(SLABS)
    S_l = np.zeros(SLABS)
    S_tp = np.zeros(SLABS)
    S_t = np.zeros(SLABS)
    S_pred = np.zeros(SLABS)
    S_xt = 0.0
    S_st = 0.0
    for r in results:
        sa = r["stats_act"].astype(np.float64)
        S_xt += r["diag_xt"].astype(np.float64).sum()
        S_st += r["diag_st"].astype(np.float64).sum()
        tr = r["trows"].astype(np.float64)
        dtp = r["diag_tp"].astype(np.float64)
        for s_i in range(SLABS):
            S_s[s_i] += sa[:, 2 * s_i].sum()
            S_l[s_i] += sa[:, 2 * s_i + 1].sum()
            S_tp[s_i] += dtp[s_i].sum()
            row = (s_i % 3) * 32
            col = 512 * (s_i // 6) + 256 * ((s_i // 3) % 2)
            S_t[s_i] += tr[row, col:col + 256].sum()
            S_pred[s_i] += tr[row, 1024 + col:1024 + col + 256].sum()

    sum_prob = N_TOTAL - S_s.sum()
    sum_pt = S_t.sum() - S_st               # sum(prob * t)
    sum_sp = -S_l.sum()                     # sum(softplus(x))
    bce = (sum_sp - S_xt) / N_TOTAL

    union = sum_prob + S_t.sum()
    inter = 2.0 * sum_pt
    dice_loss = 1.0 - (inter + EPS) / union

    score = np.where(
        (S_t == 0) & (S_pred == 0),
        np.ones_like(S_t),
        (2.0 * S_tp + EPS) / (S_t + S_pred),
    ).reshape(B, C)
    per_class = score.mean(axis=0)

    loss = (bce + dice_loss * 0.5 + per_class[0] * 0.2
            + per_class[1] * 0.1 + per_class[2] * 0.2)
    return np.float32(loss)


def kernel(logits: np.ndarray, targets: np.ndarray) -> np.ndarray:
    nc = _build()
    in_maps = _shard_inputs(np.asarray(logits), np.asarray(targets))
    res = run_bass_kernel_spmd(nc, in_maps, list(range(N_CORES)))
    return _combine(res.results)


# revision 9
# speedup vs baseline: 1.6027x; 1.0758x over previous
"""BCE + weighted Dice loss on 8 Trainium2 NeuronCores.

Full inputs logits/targets [4,3,128,128,128] f32 are sharded along the depth
axis D=128 into 8 slices of 16 and converted to bf16 on the host (halves DMA;
targets are {0,1} so exact, logits rounding shifts the loss by ~1e-5 rel).
Each core reduces its shard to per-(b,c) partial sums; the host combines them.

Math notes (s := sigmoid(-x)):
  sigmoid(x)   = 1 - s
  softplus(x)  = -ln(s)
  sum(prob)    = N - sum(s)
  sum(prob*t)  = sum(t) - sum(s*t)
  bce_sum      = -sum(ln s) - sum(x*t)
  pred         = (x >= 0.5);  t*pred = t*(x>=0.5)

Engine split per (b,c) slab tile [128, 2048] bf16:
  ScalarE: s = sigmoid(-x) (+accum sum s), later ln(s) (+accum sum ln s)
  VectorE: pred = (x >= 0.5) -> bf16 (4x mode), PSUM diag-mask extractions
  TensorE: diagonal-trick matmuls for sum(x*t), sum(s*t), sum(t*pred);
           ones-matmuls for per-slab sum(t), sum(pred)

The diagonal trick: accumulating chunk matmuls A[:,c128].T @ B[:,c128] into
one PSUM bank leaves sum_c sum_p A[p,cm]*B[p,cn] at [m,n]; the diagonal
m == n carries the elementwise dot product. Masking by the identity and
summing recovers sum(A*B) without any slow DVE reduce.

Device outputs per core:
  stats_act [128, 24]: ScalarE accums (2 cols/slab: sum s, sum ln s)
  diag_xt, diag_st [128, 128] f32: masked global-diag accumulators; their
      total sum is sum(x*t) resp. sum(s*t) over the whole shard
  diag_tp [12, 128, 128] f32: per-slab masked accumulators; total sum of
      entry s is that slab's sum(t*pred)
  trows [96, 2048] f32: ones-matmul row banks; slab s row lives at
      partition (s%3)*32, cols 1024*q + 512*(s//6) + 256*((s//3)%2) for
      quantity q in {0: sum(t), 1: sum(pred)}, 256 wide
"""

import sys

if "/opt/trn_rl_repo" not in sys.path:
    sys.path.insert(0, "/opt/trn_rl_repo")

import numpy as np

import concourse.bacc as bacc
import concourse.mybir as mybir
from concourse import tile
from concourse.alu_op_type import AluOpType
from concourse.bass_utils import run_bass_kernel_spmd

# Problem geometry (hardcoded per harness contract).
B, C, D, H, W = 4, 3, 128, 128, 128
N_CORES = 8
D_SHARD = D // N_CORES            # 16
SLABS = B * C                     # 12 (b,c) slabs per core
P = 128                           # SBUF partitions
F = D_SHARD * H * W // P          # 2048 free elems per partition
N_SLAB = P * F                    # 262144 elems per core-slab
N_TOTAL = B * C * D * H * W
NCH = F // 128                    # 16 diag chunks per slab

_CACHED = {}


def _build():
    if "nc" in _CACHED:
        return _CACHED["nc"]
    AFT = mybir.ActivationFunctionType
    f32 = mybir.dt.float32
    bf16 = mybir.dt.bfloat16

    nc = bacc.Bacc("TRN2", target_bir_lowering=False, debug=False,
                   num_devices=N_CORES)
    x_d = nc.dram_tensor("logits", [SLABS, P, F], bf16, kind="ExternalInput")
    t_d = nc.dram_tensor("targets", [SLABS, P, F], bf16, kind="ExternalInput")
    id_d = nc.dram_tensor("ident", [P, 128], bf16, kind="ExternalInput")
    sa_d = nc.dram_tensor("stats_act", [P, 2 * SLABS], f32, kind="ExternalOutput")
    dxt_d = nc.dram_tensor("diag_xt", [P, 128], f32, kind="ExternalOutput")
    dst_d = nc.dram_tensor("diag_st", [P, 128], f32, kind="ExternalOutput")
    dtp_d = nc.dram_tensor("diag_tp", [SLABS, P, 128], f32, kind="ExternalOutput")
    tr_d = nc.dram_tensor("trows", [96, 2048], f32, kind="ExternalOutput")

    with tile.TileContext(nc) as tc:
        with (
            tc.tile_pool(name="xt", bufs=6) as xt_pool,
            tc.tile_pool(name="s", bufs=SLABS) as s_pool,
            tc.tile_pool(name="scr", bufs=2) as scr_pool,
            tc.tile_pool(name="misc", bufs=1) as misc_pool,
            tc.tile_pool(name="psum", bufs=1, space="PSUM") as psum_pool,
        ):
            stats_act = misc_pool.tile([P, 2 * SLABS], f32)
            nc.vector.memset(stats_act[:], 0.0)
            ones = misc_pool.tile([P, 1], bf16)
            nc.vector.memset(ones[:], 1.0)
            ident = misc_pool.tile([P, 128], bf16)
            nc.sync.dma_start(ident[:], id_d[:])

            # PSUM banks (8 total): 2 global diag accumulators, 2 rotating
            # per-slab tp accumulators, 2+2 row banks for sum(t)/sum(pred).
            p_xt = psum_pool.tile([P, 128], f32, name="p_xt", tag="p_xt")
            p_st = psum_pool.tile([P, 128], f32, name="p_st", tag="p_st")
            p_tp = [psum_pool.tile([P, 128], f32, name=f"p_tp{i}", tag=f"p_tp{i}")
                    for i in range(2)]
            p_t = [psum_pool.tile([P, 512], f32, name=f"p_t{i}", tag=f"p_t{i}")
                   for i in range(2)]
            p_pr = [psum_pool.tile([P, 512], f32, name=f"p_pr{i}", tag=f"p_pr{i}")
                    for i in range(2)]

            s_tiles = []
            xb_tiles = [None] * SLABS

            def issue_x(j):
                xb_tiles[j] = xt_pool.tile([P, F], bf16, tag="x",
                                           name=f"xb{j}")
                nc.sync.dma_start(xb_tiles[j][:], x_d[j])

            # Lead the x stream two slabs ahead of t so the sigmoid chain
            # (the ScalarE critical path) is not gated by the tail of the
            # interleaved DMA stream.
            issue_x(0)
            issue_x(1)

            # ScalarE runs in G groups of sigmoid-then-ln so ln work overlaps
            # the DMA stream instead of serializing after the last sigmoid.
            # zb tiles chain the groups so the activation-table set switches
            # exactly 2*G-1 times.
            GROUPS = 3
            GSIZE = SLABS // GROUPS
            act_gate = [None]       # bias AP for the next sigmoid group
            last_l = [None]

            # ---- Phase A (sigmoid table set resident) ----
            for s_i in range(SLABS):
                if s_i + 2 < SLABS:
                    issue_x(s_i + 2)
                xb = xb_tiles[s_i]
                tb = xt_pool.tile([P, F], bf16, tag="t")
                nc.sync.dma_start(tb[:], t_d[s_i])

                # s = sigmoid(-x) (+ accum sum s)
                s = s_pool.tile([P, F], bf16, tag="s")
                s_tiles.append(s)
                sig_bias = act_gate[0][:] if act_gate[0] is not None else 0.0
                nc.scalar.activation(
                    s[:], xb[:], AFT.Sigmoid, scale=-1.0, bias=sig_bias,
                    accum_out=stats_act[:, 2 * s_i:2 * s_i + 1],
                )
                if s_i % GSIZE == GSIZE - 1:
                    # End of sigmoid group: run this group's ln passes.
                    g0 = s_i - (GSIZE - 1)
                    zb = misc_pool.tile([P, 1], f32, name=f"zbA{s_i}",
                                        tag=f"zbA{s_i}")
                    nc.scalar.activation(zb[:], s[:, 0:1], AFT.Copy, scale=0.0)
                    for k in range(g0, s_i + 1):
                        l = scr_pool.tile([P, F], bf16, tag="l", name=f"l{k}")
                        nc.scalar.activation(
                            l[:], s_tiles[k][:], AFT.Ln, bias=zb[:],
                            accum_out=stats_act[:, 2 * k + 1:2 * k + 2],
                        )
                        last_l[0] = l
                    if s_i != SLABS - 1:
                        zb2 = misc_pool.tile([P, 1], f32, name=f"zbB{s_i}",
                                             tag=f"zbB{s_i}")
                        nc.scalar.activation(zb2[:], last_l[0][:, 0:1],
                                             AFT.Copy, scale=0.0)
                        act_gate[0] = zb2
                # pred = (x >= 0.5) in bf16 (4x mode)
                pred = scr_pool.tile([P, F], bf16, tag="pred")
                nc.vector.tensor_scalar(
                    out=pred[:], in0=xb[:], scalar1=0.5, scalar2=None,
                    op0=AluOpType.is_ge,
                )

                first = s_i == 0
                last = s_i == SLABS - 1
                for c in range(NCH):
                    sl = slice(c * 128, (c + 1) * 128)
                    nc.tensor.matmul(p_xt[:, :], xb[:, sl], tb[:, sl],
                                     start=(first and c == 0),
                                     stop=(last and c == NCH - 1))
                for c in range(NCH):
                    sl = slice(c * 128, (c + 1) * 128)
                    nc.tensor.matmul(p_st[:, :], s[:, sl], tb[:, sl],
                                     start=(first and c == 0),
                                     stop=(last and c == NCH - 1))
                tp_bank = p_tp[s_i % 2]
                for c in range(NCH):
                    sl = slice(c * 128, (c + 1) * 128)
                    nc.tensor.matmul(tp_bank[:, :], pred[:, sl], tb[:, sl],
                                     start=(c == 0), stop=(c == NCH - 1))
                # Per-slab extraction of the tp diagonal (masked, f32).
                mtp = scr_pool.tile([P, 128], f32, tag="mtp")
                nc.vector.tensor_tensor(out=mtp[:], in0=tp_bank[:, :],
                                        in1=ident[:], op=AluOpType.mult)
                nc.gpsimd.dma_start(dtp_d[s_i], mtp[:])

                # Row sums: sum(t) and sum(pred) per slab via ones-matmuls,
                # 8 chunks of 256 accumulated into one [1,256] row slot.
                row = (s_i % 3) * 32
                colblk = ((s_i // 3) % 2) * 256
                t_bank = p_t[s_i // 6]
                pr_bank = p_pr[s_i // 6]
                for c in range(8):
                    sl = slice(c * 256, (c + 1) * 256)
                    nc.tensor.matmul(t_bank[row:row + 1, colblk:colblk + 256],
                                     ones[:], tb[:, sl],
                                     start=(c == 0), stop=(c == 7))
                for c in range(8):
                    sl = slice(c * 256, (c + 1) * 256)
                    nc.tensor.matmul(pr_bank[row:row + 1, colblk:colblk + 256],
                                     ones[:], pred[:, sl],
                                     start=(c == 0), stop=(c == 7))


            # ---- Epilogue ----
            mxt = misc_pool.tile([P, 128], f32)
            nc.vector.tensor_tensor(out=mxt[:], in0=p_xt[:, :], in1=ident[:],
                                    op=AluOpType.mult)
            nc.sync.dma_start(dxt_d[:], mxt[:])
            mst = misc_pool.tile([P, 128], f32)
            nc.vector.tensor_tensor(out=mst[:], in0=p_st[:, :], in1=ident[:],
                                    op=AluOpType.mult)
            nc.sync.dma_start(dst_d[:], mst[:])

            trows = misc_pool.tile([96, 2048], f32)
            for i in range(2):
                nc.vector.tensor_copy(trows[0:96, 512 * i:512 * (i + 1)],
                                      p_t[i][0:96, :])
                nc.vector.tensor_copy(trows[0:96, 1024 + 512 * i:1024 + 512 * (i + 1)],
                                      p_pr[i][0:96, :])
            nc.sync.dma_start(tr_d[:], trows[:])
            nc.sync.dma_start(sa_d[:], stats_act[:])

    nc.compile()
    _CACHED["nc"] = nc
    return nc


def _to_bf16_bits(a: np.ndarray) -> np.ndarray:
    """f32 -> bf16 bits with round-to-nearest-even, returned as uint16."""
    u = np.ascontiguousarray(a, dtype=np.float32).view(np.uint32)
    rounded = ((u + 0x7FFF + ((u >> 16) & 1)) >> 16).astype(np.uint16)
    return rounded


def _shard_inputs(logits: np.ndarray, targets: np.ndarray):
    import ml_dtypes

    bf = ml_dtypes.bfloat16
    xb = _to_bf16_bits(logits).view(bf)
    tb = _to_bf16_bits(targets).view(bf)
    eye = np.eye(P, 128, dtype=np.float32).astype(bf)
    in_maps = []
    for i in range(N_CORES):
        sl = slice(i * D_SHARD, (i + 1) * D_SHARD)
        x = np.ascontiguousarray(xb[:, :, sl]).reshape(SLABS, P, F)
        t = np.ascontiguousarray(tb[:, :, sl]).reshape(SLABS, P, F)
        in_maps.append({"logits": x, "targets": t, "ident": eye})
    return in_maps


def _combine(results):
    """Host-side reduction of per-core partials to the scalar loss."""
    EPS = 1e-9
    S_s = np.zeros(SLABS)
    S_l = np.zeros(SLABS)
    S_tp = np.zeros(SLABS)
    S_t = np.zeros(SLABS)
    S_pred = np.zeros(SLABS)
    S_xt = 0.0
    S_st = 0.0
    for r in results:
        sa = r["stats_act"].astype(np.float64)
        S_xt += r["diag_xt"].astype(np.float64).sum()
        S_st += r["diag_st"].astype(np.float64).sum()
        tr = r["trows"].astype(np.float64)
        dtp = r["diag_tp"].astype(np.float64)
        for s_i in range(SLABS):
            S_s[s_i] += sa[:, 2 * s_i].sum()
            S_l[s_i] += sa[:, 2 * s_i + 1].sum()
            S_tp[s_i] += dtp[s_i].sum()
            row = (s_i % 3) * 32
            col = 512 * (s_i // 6) + 256 * ((s_i // 3) % 2)
            S_t[s_i] += tr[row, col:col + 256].sum()
            S_pred[s_i] += tr[row, 1024 + col:1024 + col + 256].sum()

    sum_prob = N_TOTAL - S_s.sum()
    sum_pt = S_t.sum() - S_st               # sum(prob * t)
    sum_sp = -S_l.sum()                     # sum(softplus(x))
    bce = (sum_sp - S_xt) / N_TOTAL

    union = sum_prob + S_t.sum()
    inter = 2.0 * sum_pt
    dice_loss = 1.0 - (inter + EPS) / union

    score = np.where(
        (S_t == 0) & (S_pred == 0),
        np.ones_like(S_t),
        (2.0 * S_tp + EPS) / (S_t + S_pred),
    ).reshape(B, C)
    per_class = score.mean(axis=0)

    loss = (bce + dice_loss * 0.5 + per_class[0] * 0.2
            + per_class[1] * 0.1 + per_class[2] * 0.2)
    return np.float32(loss)


def kernel(logits: np.ndarray, targets: np.ndarray) -> np.ndarray:
    nc = _build()
    in_maps = _shard_inputs(np.asarray(logits), np.asarray(targets))
    res = run_bass_kernel_spmd(nc, in_maps, list(range(N_CORES)))
    return _combine(res.results)


# revision 10
# speedup vs baseline: 1.7410x; 1.0863x over previous
"""BCE + weighted Dice loss on 8 Trainium2 NeuronCores.

Full inputs logits/targets [4,3,128,128,128] f32 are sharded along the depth
axis D=128 into 8 slices of 16 and converted to bf16 on the host (halves DMA;
targets are {0,1} so exact, logits rounding shifts the loss by ~1e-5 rel).
Each core reduces its shard to per-(b,c) partial sums; the host combines them.

Math notes (s := sigmoid(-x)):
  sigmoid(x)   = 1 - s
  softplus(x)  = -ln(s)
  sum(prob)    = N - sum(s)
  sum(prob*t)  = sum(t) - sum(s*t)
  bce_sum      = -sum(ln s) - sum(x*t)
  pred         = (x >= 0.5);  t*pred = t*(x>=0.5)

Work is organized in 3 "quads" of 4 (b,c) slabs, [128, 8192] tiles, so the
per-op fixed costs (ScalarE 352-cycle ramp, DVE drains, semaphores) amortize.
Global sums (sum s, sum ln s, sum x*t) accumulate per quad; per-(b,c) sums
(sum t, sum pred, sum t*pred) are produced per 2048-column slab slice.

Engine split:
  ScalarE: s = sigmoid(-x) (+accum), ln(s) (+accum), alternating per quad,
      chained via zero-bias tiles so the activation table set loads 6x total
  VectorE: pred = (x >= 0.5) (bf16 4x mode), sum(x*t) via fused
      scalar_tensor_tensor accumulate, PSUM diag-mask extractions
  TensorE: diagonal-trick matmuls for sum(s*t) (global) and sum(t*pred)
      (per slab); ones-matmuls for per-slab sum(t) / sum(pred)

The diagonal trick: accumulating chunk matmuls A[:,c128].T @ B[:,c128] into
one PSUM bank leaves sum_c sum_p A[p,cm]*B[p,cn] at [m,n]; the diagonal
m == n carries the elementwise dot product. Masking by the identity (a tiny
host-supplied input) recovers sum(A*B) without any slow DVE reduce.

Device outputs per core:
  stats_act [128, 6]: ScalarE accums per quad (sum s, sum ln s)
  stats_dve [128, 3]: VectorE accums per quad (sum x*t)
  diag_st [128, 128] f32: masked global-diag accumulator -> sum(s*t)
  diag_tp [12, 128, 128] f32: per-slab masked accumulators -> sum(t*pred)
  trows [96, 2048] f32: ones-matmul row banks; slab s row lives at
      partition (s%3)*32, cols 1024*q + 512*(s//6) + 256*((s//3)%2) for
      quantity q in {0: sum(t), 1: sum(pred)}, 256 wide
"""

import sys

if "/opt/trn_rl_repo" not in sys.path:
    sys.path.insert(0, "/opt/trn_rl_repo")

import numpy as np

import concourse.bacc as bacc
import concourse.mybir as mybir
from concourse import tile
from concourse.alu_op_type import AluOpType
from concourse.bass_utils import run_bass_kernel_spmd

# Problem geometry (hardcoded per harness contract).
B, C, D, H, W = 4, 3, 128, 128, 128
N_CORES = 8
D_SHARD = D // N_CORES            # 16
SLABS = B * C                     # 12 (b,c) slabs per core
P = 128                           # SBUF partitions
F = D_SHARD * H * W // P          # 2048 free elems per slab per partition
N_SLAB = P * F                    # 262144 elems per core-slab
N_TOTAL = B * C * D * H * W
QUADS = 3
QS = SLABS // QUADS               # 4 slabs per quad
QF = QS * F                       # 8192 free elems per quad tile

_CACHED = {}


def _build():
    if "nc" in _CACHED:
        return _CACHED["nc"]
    AFT = mybir.ActivationFunctionType
    f32 = mybir.dt.float32
    bf16 = mybir.dt.bfloat16

    nc = bacc.Bacc("TRN2", target_bir_lowering=False, debug=False,
                   num_devices=N_CORES)
    x_d = nc.dram_tensor("logits", [QUADS, P, QF], bf16, kind="ExternalInput")
    t_d = nc.dram_tensor("targets", [QUADS, P, QF], bf16, kind="ExternalInput")
    id_d = nc.dram_tensor("ident", [P, 128], bf16, kind="ExternalInput")
    sa_d = nc.dram_tensor("stats_act", [P, 2 * QUADS], f32, kind="ExternalOutput")
    sd_d = nc.dram_tensor("stats_dve", [P, QUADS], f32, kind="ExternalOutput")
    dst_d = nc.dram_tensor("diag_st", [P, 128], f32, kind="ExternalOutput")
    dtp_d = nc.dram_tensor("diag_tp", [SLABS, P, 128], f32, kind="ExternalOutput")
    tr_d = nc.dram_tensor("trows", [96, 2048], f32, kind="ExternalOutput")

    with tile.TileContext(nc) as tc:
        with (
            tc.tile_pool(name="xt", bufs=2) as xt_pool,
            tc.tile_pool(name="s", bufs=QUADS) as s_pool,
            tc.tile_pool(name="pred", bufs=2) as pred_pool,
            tc.tile_pool(name="scr", bufs=2) as scr_pool,
            tc.tile_pool(name="misc", bufs=1) as misc_pool,
            tc.tile_pool(name="psum", bufs=1, space="PSUM") as psum_pool,
        ):
            stats_act = misc_pool.tile([P, 2 * QUADS], f32)
            nc.vector.memset(stats_act[:], 0.0)
            stats_dve = misc_pool.tile([P, QUADS], f32)
            nc.vector.memset(stats_dve[:], 0.0)
            ones = misc_pool.tile([P, 1], bf16)
            nc.vector.memset(ones[:], 1.0)
            ident = misc_pool.tile([P, 128], bf16)
            nc.sync.dma_start(ident[:], id_d[:])

            # PSUM banks (7 of 8): global st diag, 2 rotating tp diags,
            # 2+2 row banks for sum(t)/sum(pred).
            p_st = psum_pool.tile([P, 128], f32, name="p_st", tag="p_st")
            p_tp = [psum_pool.tile([P, 128], f32, name=f"p_tp{i}", tag=f"p_tp{i}")
                    for i in range(2)]
            p_t = [psum_pool.tile([P, 512], f32, name=f"p_t{i}", tag=f"p_t{i}")
                   for i in range(2)]
            p_pr = [psum_pool.tile([P, 512], f32, name=f"p_pr{i}", tag=f"p_pr{i}")
                    for i in range(2)]

            act_gate = None
            for q in range(QUADS):
                xq = xt_pool.tile([P, QF], bf16, tag="x", name=f"xq{q}")
                nc.sync.dma_start(xq[:], x_d[q])
                tq = xt_pool.tile([P, QF], bf16, tag="t", name=f"tq{q}")
                nc.sync.dma_start(tq[:], t_d[q])

                # s = sigmoid(-x) (+ accum sum s for the quad)
                sq = s_pool.tile([P, QF], bf16, tag="s", name=f"sq{q}")
                nc.scalar.activation(
                    sq[:], xq[:], AFT.Sigmoid, scale=-1.0,
                    bias=(act_gate[:] if act_gate is not None else 0.0),
                    accum_out=stats_act[:, 2 * q:2 * q + 1],
                )
                # pred = (x >= 0.5) in bf16 (4x mode)
                pq = pred_pool.tile([P, QF], bf16, tag="pred", name=f"pq{q}")
                nc.vector.tensor_scalar(
                    out=pq[:], in0=xq[:], scalar1=0.5, scalar2=None,
                    op0=AluOpType.is_ge,
                )
                # sum(x*t) for the quad via fused STT accumulate
                uq = scr_pool.tile([P, QF], bf16, tag="u", name=f"uq{q}")
                nc.vector.scalar_tensor_tensor(
                    out=uq[:], in0=xq[:], scalar=1.0, in1=tq[:],
                    op0=AluOpType.mult, op1=AluOpType.mult,
                    accum_out=stats_dve[:, q:q + 1],
                )

                # Per-slab PE reductions.
                for j in range(QS):
                    s_i = q * QS + j
                    base = j * F
                    first = s_i == 0
                    last = s_i == SLABS - 1
                    for c in range(16):
                        sl = slice(base + c * 128, base + (c + 1) * 128)
                        nc.tensor.matmul(p_st[:, :], sq[:, sl], tq[:, sl],
                                         start=(first and c == 0),
                                         stop=(last and c == 15))
                    tp_bank = p_tp[s_i % 2]
                    for c in range(16):
                        sl = slice(base + c * 128, base + (c + 1) * 128)
                        nc.tensor.matmul(tp_bank[:, :], pq[:, sl], tq[:, sl],
                                         start=(c == 0), stop=(c == 15))
                    mtp = scr_pool.tile([P, 128], f32, tag="mtp",
                                        name=f"mtp{s_i}")
                    nc.vector.tensor_tensor(out=mtp[:], in0=tp_bank[:, :],
                                            in1=ident[:], op=AluOpType.mult)
                    nc.gpsimd.dma_start(dtp_d[s_i], mtp[:])

                    row = (s_i % 3) * 32
                    colblk = ((s_i // 3) % 2) * 256
                    t_bank = p_t[s_i // 6]
                    pr_bank = p_pr[s_i // 6]
                    for c in range(8):
                        sl = slice(base + c * 256, base + (c + 1) * 256)
                        nc.tensor.matmul(
                            t_bank[row:row + 1, colblk:colblk + 256],
                            ones[:], tq[:, sl], start=(c == 0), stop=(c == 7))
                    for c in range(8):
                        sl = slice(base + c * 256, base + (c + 1) * 256)
                        nc.tensor.matmul(
                            pr_bank[row:row + 1, colblk:colblk + 256],
                            ones[:], pq[:, sl], start=(c == 0), stop=(c == 7))

                # ln(s) for this quad (+ accum), gated behind this quad's
                # sigmoid via a zero bias tile so table set switches are
                # exactly one per quad boundary.
                zba = misc_pool.tile([P, 1], f32, name=f"zbA{q}", tag=f"zbA{q}")
                nc.scalar.activation(zba[:], sq[:, 0:1], AFT.Copy, scale=0.0)
                lq = scr_pool.tile([P, QF], bf16, tag="l", bufs=1, name=f"lq{q}")
                nc.scalar.activation(
                    lq[:], sq[:], AFT.Ln, bias=zba[:],
                    accum_out=stats_act[:, 2 * q + 1:2 * q + 2],
                )
                if q != QUADS - 1:
                    zbb = misc_pool.tile([P, 1], f32, name=f"zbB{q}",
                                         tag=f"zbB{q}")
                    nc.scalar.activation(zbb[:], lq[:, 0:1], AFT.Copy,
                                         scale=0.0)
                    act_gate = zbb

            # ---- Epilogue ----
            mst = misc_pool.tile([P, 128], f32)
            nc.vector.tensor_tensor(out=mst[:], in0=p_st[:, :], in1=ident[:],
                                    op=AluOpType.mult)
            nc.sync.dma_start(dst_d[:], mst[:])

            trows = misc_pool.tile([96, 2048], f32)
            for i in range(2):
                nc.vector.tensor_copy(trows[0:96, 512 * i:512 * (i + 1)],
                                      p_t[i][0:96, :])
                nc.vector.tensor_copy(trows[0:96, 1024 + 512 * i:1024 + 512 * (i + 1)],
                                      p_pr[i][0:96, :])
            nc.sync.dma_start(tr_d[:], trows[:])
            nc.sync.dma_start(sa_d[:], stats_act[:])
            nc.sync.dma_start(sd_d[:], stats_dve[:])

    nc.compile()
    _CACHED["nc"] = nc
    return nc


def _to_bf16_bits(a: np.ndarray) -> np.ndarray:
    """f32 -> bf16 bits with round-to-nearest-even, returned as uint16."""
    u = np.ascontiguousarray(a, dtype=np.float32).view(np.uint32)
    rounded = ((u + 0x7FFF + ((u >> 16) & 1)) >> 16).astype(np.uint16)
    return rounded


def _shard_inputs(logits: np.ndarray, targets: np.ndarray):
    import ml_dtypes

    bf = ml_dtypes.bfloat16
    xb = _to_bf16_bits(logits).view(bf)
    tb = _to_bf16_bits(targets).view(bf)
    eye = np.eye(P, 128, dtype=np.float32).astype(bf)
    in_maps = []
    for i in range(N_CORES):
        sl = slice(i * D_SHARD, (i + 1) * D_SHARD)
        x = np.ascontiguousarray(xb[:, :, sl]).reshape(QUADS, P, QF)
        t = np.ascontiguousarray(tb[:, :, sl]).reshape(QUADS, P, QF)
        in_maps.append({"logits": x, "targets": t, "ident": eye})
    return in_maps


def _combine(results):
    """Host-side reduction of per-core partials to the scalar loss."""
    EPS = 1e-9
    S_tp = np.zeros(SLABS)
    S_t = np.zeros(SLABS)
    S_pred = np.zeros(SLABS)
    S_s = 0.0
    S_l = 0.0
    S_xt = 0.0
    S_st = 0.0
    for r in results:
        sa = r["stats_act"].astype(np.float64)
        S_s += sa[:, 0::2].sum()
        S_l += sa[:, 1::2].sum()
        S_xt += r["stats_dve"].astype(np.float64).sum()
        S_st += r["diag_st"].astype(np.float64).sum()
        tr = r["trows"].astype(np.float64)
        dtp = r["diag_tp"].astype(np.float64)
        for s_i in range(SLABS):
            S_tp[s_i] += dtp[s_i].sum()
            row = (s_i % 3) * 32
            col = 512 * (s_i // 6) + 256 * ((s_i // 3) % 2)
            S_t[s_i] += tr[row, col:col + 256].sum()
            S_pred[s_i] += tr[row, 1024 + col:1024 + col + 256].sum()

    sum_prob = N_TOTAL - S_s
    sum_pt = S_t.sum() - S_st               # sum(prob * t)
    sum_sp = -S_l                           # sum(softplus(x))
    bce = (sum_sp - S_xt) / N_TOTAL

    union = sum_prob + S_t.sum()
    inter = 2.0 * sum_pt
    dice_loss = 1.0 - (inter + EPS) / union

    score = np.where(
        (S_t == 0) & (S_pred == 0),
        np.ones_like(S_t),
        (2.0 * S_tp + EPS) / (S_t + S_pred),
    ).reshape(B, C)
    per_class = score.mean(axis=0)

    loss = (bce + dice_loss * 0.5 + per_class[0] * 0.2
            + per_class[1] * 0.1 + per_class[2] * 0.2)
    return np.float32(loss)


def kernel(logits: np.ndarray, targets: np.ndarray) -> np.ndarray:
    nc = _build()
    in_maps = _shard_inputs(np.asarray(logits), np.asarray(targets))
    res = run_bass_kernel_spmd(nc, in_maps, list(range(N_CORES)))
    return _combine(res.results)
